# revision 14
# baseline (speedup 1.0000x reference)
"""Self-contained Trainium2 kernel for nn_Attention_12240656794051.

kernel(**inputs) takes FULL unsharded numpy inputs (as in setup_inputs()) and
returns the FULL [8, 4097, 768] float32 output.  Sharding: data-parallel over
batch B=8 -> one sample per NeuronCore (windows are independent per sample),
no collectives.

Per-core program (bf16 compute, f32 accumulate):
  - host ships x/lp pre-transposed to feature-major window-major stripes, so
    the device does no input transposes.
  - QKV / prefix-KV projections as PE GEMMs (weight-stationary for Q/K,
    data-stationary for V); Q pre-scaled by 1/sqrt(hd) on host (exact, /8).
  - windowed attention: scores per (window-pair, head) stacked [128q, 128k];
    masked_fill emulated as min(s, M) with M = 2e4*tq*tk - 30 built by a
    K=3 matmul.  Scores are bounded (|s| < ~4) so softmax needs no max
    subtraction: exp(min(s, M)) with accum_out rowsum; fully-masked rows
    give uniform 1/128 exactly (128*e^-30 normalized).
  - global aggregation (NC=1 cls token): host precomputes wsg = Wk @ (q_cls
    * scale) per head, so global scores sg = wsg^T @ x fuse into phase A as
    one 6-matmul group per stripe.  Softmax normalization is folded into the
    final [12,768] psum evict (per-partition scale).
  - output projection fused per stripe (no xiT spill); the global-broadcast
    stage softmaxes over a single key so it reduces to x_img += v_cls, which
    the HOST applies (corr row + bias + cls-row projection are tiny f32
    matmuls on host using the exact f32 weights).

Algebraic facts used (validated vs reference): channel-broadcast token mask
makes amask the exact outer product tq x tk and key_strength == tok; b_proj
and the v_cls broadcast fold into a rank-1 correction of the projection.
"""

import os
import sys

if "/opt/trn_rl_repo" not in sys.path:
    sys.path.insert(0, "/opt/trn_rl_repo")
os.environ.setdefault("JAX_PLATFORMS", "axon")

from contextlib import ExitStack

import numpy as np
import ml_dtypes

import concourse.bass as bass
import concourse.bacc as bacc
import concourse.tile as tile
from concourse import mybir
from concourse.masks import make_identity

BF16 = mybir.dt.bfloat16
F32 = mybir.dt.float32
AF = mybir.ActivationFunctionType
ALU = mybir.AluOpType

NSTRIPES = int(os.environ.get("K_STRIPES", "8"))
PHASES = os.environ.get("K_PHASES", "ab")

B = 8
C = 768
NH = 12
HD = 64
L = 4096
N = L + 1
CH = 6          # feature chunks of 128
ST = 8          # stripes
SW = 512        # tokens per stripe
WPS = 8         # windows per stripe
SCALE = HD ** -0.5


def build_nc():
    nc = bacc.Bacc()

    xT_d = nc.declare_dram_parameter("xT", [128, ST * CH * SW], BF16, isOutput=False)
    lpT_d = nc.declare_dram_parameter("lpT", [128, ST * CH * SW], BF16, isOutput=False)
    Mt_d = nc.declare_dram_parameter("Mt_all", [128, 32 * 128], BF16, isOutput=False)
    mg_d = nc.declare_dram_parameter("mg", [NH, L], BF16, isOutput=False)
    wsg_d = nc.declare_dram_parameter("wsg", [128, CH * NH], BF16, isOutput=False)
    w_qkv_d = nc.declare_dram_parameter("w_qkv", [C, 3 * C], BF16, isOutput=False)
    w_kvp_d = nc.declare_dram_parameter("w_kvp", [C, 2 * C], BF16, isOutput=False)
    wproj_d = nc.declare_dram_parameter("wproj", [C, C], BF16, isOutput=False)
    out_img_d = nc.declare_dram_parameter("out_img", [L, C], F32, isOutput=True)
    cls_att_d = nc.declare_dram_parameter("cls_att", [1, C], F32, isOutput=True)

    v_spill = nc.dram_tensor("v_spill", [L, C], BF16)

    with tile.TileContext(nc) as tc, ExitStack() as ctx:
        pw = ctx.enter_context(tc.tile_pool(name="pw", bufs=1))
        pstripe = ctx.enter_context(tc.tile_pool(name="pstripe", bufs=1))
        psm = ctx.enter_context(tc.tile_pool(name="psm", bufs=3))
        pB = ctx.enter_context(tc.tile_pool(name="pB", bufs=1))
        ps_g = ctx.enter_context(tc.tile_pool(name="ps_g", bufs=2, space="PSUM"))
        ps_sc = ctx.enter_context(tc.tile_pool(name="ps_sc", bufs=3, space="PSUM"))
        ps_pt = ctx.enter_context(tc.tile_pool(name="ps_pt", bufs=2, space="PSUM"))
        ps_xw = ctx.enter_context(tc.tile_pool(name="ps_xw", bufs=1, space="PSUM"))
        py = ctx.enter_context(tc.tile_pool(name="py", bufs=3))

        # ---------------- prologue ----------------
        ident = pw.tile([128, 128], BF16, tag="ident")
        make_identity(nc, ident[:])

        # weight tiles; DMAs emitted in first-use order so stripe 0 can
        # start as soon as its operands land (startup-latency hiding)
        w_qkv = pw.tile([128, CH * 3 * C], BF16, tag="w_qkv")
        w_kvp = pw.tile([128, CH * 2 * C], BF16, tag="w_kvp")
        wproj = pw.tile([128, CH * C], BF16, tag="wproj")
        wsg = pw.tile([128, CH * NH], BF16, tag="wsg")
        nc.sync.dma_start(wsg[:], wsg_d[:])
        for blk in range(2):                       # Q then K chunks
            for c in range(CH):
                nc.sync.dma_start(
                    w_qkv[:, c * 3 * C + blk * C: c * 3 * C + (blk + 1) * C],
                    w_qkv_d[c * 128:(c + 1) * 128, blk * C:(blk + 1) * C])
        for c in range(CH):                        # K_pre
            nc.sync.dma_start(w_kvp[:, c * 2 * C: c * 2 * C + C],
                              w_kvp_d[c * 128:(c + 1) * 128, 0:C])
        for c in range(CH):                        # V
            nc.sync.dma_start(w_qkv[:, c * 3 * C + 2 * C: (c + 1) * 3 * C],
                              w_qkv_d[c * 128:(c + 1) * 128, 2 * C:3 * C])
        for c in range(CH):                        # V_pre
            nc.sync.dma_start(w_kvp[:, c * 2 * C + C: (c + 1) * 2 * C],
                              w_kvp_d[c * 128:(c + 1) * 128, C:2 * C])
        for c in range(CH):                        # proj
            nc.sync.dma_start(wproj[:, c * C:(c + 1) * C],
                              wproj_d[c * 128:(c + 1) * 128, :])
        mg_sb = pw.tile([NH, L], BF16, tag="mg")
        nc.sync.dma_start(mg_sb[:], mg_d[:])

        pg_full = pw.tile([NH, L], BF16, tag="pg")     # global exp weights
        pgT_all = pw.tile([128, 32 * NH], BF16, tag="pgT")

        # ---------------- phase A: per-stripe everything ----------------
        for s in range(NSTRIPES):
            xT = pstripe.tile([128, CH * SW], BF16, tag="xT", bufs=2)
            lpT = pstripe.tile([128, CH * SW], BF16, tag="lpT", bufs=2)
            nc.sync.dma_start(xT[:], xT_d[:, s * CH * SW:(s + 1) * CH * SW])
            nc.sync.dma_start(lpT[:], lpT_d[:, s * CH * SW:(s + 1) * CH * SW])
            Mt_sb = pstripe.tile([128, 512], BF16, tag="Mt", bufs=2)
            nc.sync.dma_start(Mt_sb[:], Mt_d[:, s * 512:(s + 1) * 512])

            # global scores sg = wsg^T @ x; exp (no max needed) -> pg
            sgp = ps_g.tile([NH, SW], F32, tag="g")
            for c in range(CH):
                nc.tensor.matmul(sgp[:], wsg[:, c * NH:(c + 1) * NH],
                                 xT[:, c * SW:(c + 1) * SW],
                                 start=(c == 0), stop=(c == CH - 1))
            sgn = psm.tile([NH, SW], BF16, tag="sgn", bufs=2)
            nc.vector.tensor_tensor(out=sgn[:], in0=sgp[:],
                                    in1=mg_sb[:, s * SW:(s + 1) * SW],
                                    op=ALU.min)
            nc.scalar.activation(pg_full[:, s * SW:(s + 1) * SW], sgn[:],
                                 AF.Exp, scale=1.0)
            for tt in range(4):
                gt = s * 4 + tt
                tp = ps_pt.tile([128, NH], BF16, tag="tp")
                nc.tensor.transpose(
                    tp[:], pg_full[:, s * SW + tt * 128: s * SW + (tt + 1) * 128],
                    ident[0:NH, 0:NH])
                if tt % 2 == 0:
                    nc.scalar.copy(pgT_all[:, gt * NH:(gt + 1) * NH], tp[:])
                else:
                    nc.vector.tensor_copy(pgT_all[:, gt * NH:(gt + 1) * NH], tp[:])

            # Q gemm -> qT (feature-major; w_q pre-scaled on host)
            qT = pstripe.tile([128, CH * SW], BF16, tag="qT", bufs=2)
            for m in range(CH):
                ps = ps_g.tile([128, SW], F32, tag="g")
                for c in range(CH):
                    nc.tensor.matmul(
                        ps[:],
                        w_qkv[:, c * 3 * C + m * 128: c * 3 * C + (m + 1) * 128],
                        xT[:, c * SW:(c + 1) * SW],
                        start=(c == 0), stop=(c == CH - 1))
                nc.scalar.copy(qT[:, m * SW:(m + 1) * SW], ps[:])

            # K / K_pre gemms -> kw_all (per window w, 768 cols: chunk m has
            # 64 pre keys then 64 img keys)
            kw_all = pstripe.tile([128, WPS * 768], BF16, tag="kw_all", bufs=2)
            kwr = kw_all[:, :].rearrange("p (w x) -> p w x", x=768)
            for m in range(CH):
                ps = ps_g.tile([128, SW], F32, tag="g")
                for c in range(CH):
                    nc.tensor.matmul(
                        ps[:],
                        w_qkv[:, c * 3 * C + C + m * 128: c * 3 * C + C + (m + 1) * 128],
                        xT[:, c * SW:(c + 1) * SW],
                        start=(c == 0), stop=(c == CH - 1))
                psr = ps[:, :].rearrange("p (w k) -> p w k", k=64)
                nc.scalar.copy(kwr[:, :, m * 128 + 64: m * 128 + 128], psr[:])
            for m in range(CH):
                ps = ps_g.tile([128, SW], F32, tag="g")
                for c in range(CH):
                    nc.tensor.matmul(
                        ps[:],
                        w_kvp[:, c * 2 * C + m * 128: c * 2 * C + (m + 1) * 128],
                        lpT[:, c * SW:(c + 1) * SW],
                        start=(c == 0), stop=(c == CH - 1))
                psr = ps[:, :].rearrange("p (w k) -> p w k", k=64)
                nc.scalar.copy(kwr[:, :, m * 128: m * 128 + 64], psr[:])

            # V gemms (token-major).  vw_all: per window (780 cols) per head
            # (65 cols): 64 V columns + a ones column -> AV emits row-sums.
            # The global AV also consumes vtmp directly (no DRAM spill).
            vw_all = pstripe.tile([128, WPS * 780], BF16, tag="vw_all", bufs=2)
            ones_ap = vw_all[:, :].rearrange("p (wh x) -> p wh x", x=65)
            nc.vector.memset(ones_ap[:, :, 64:65], 1.0)
            vwr = vw_all[:, :].rearrange("p (w h x) -> p w h x", h=NH, x=65)
            for tt in range(4):
                for half in range(2):
                    ps = ps_g.tile([128, 384], F32, tag="g")
                    for c in range(CH):
                        nc.tensor.matmul(
                            ps[:],
                            xT[:, c * SW + tt * 128: c * SW + (tt + 1) * 128],
                            w_qkv[:, c * 3 * C + 2 * C + half * 384: c * 3 * C + 2 * C + (half + 1) * 384],
                            start=(c == 0), stop=(c == CH - 1))
                    vtmp = psm.tile([128, 384], BF16, tag="vtmp", bufs=6)
                    nc.scalar.copy(vtmp[:], ps[:])
                    for wl in range(2):
                        w = tt * 2 + wl
                        nc.gpsimd.dma_start(
                            vwr[64:128, w, half * 6:(half + 1) * 6, 0:64],
                            vtmp[wl * 64:(wl + 1) * 64, :].rearrange(
                                "p (h x) -> p h x", x=64))
                    gt = s * SW + tt * 128
                    nc.gpsimd.dma_start(
                        v_spill[gt:gt + 128, half * 384:(half + 1) * 384], vtmp[:])
                for half in range(2):
                    ps = ps_g.tile([128, 384], F32, tag="g")
                    for c in range(CH):
                        nc.tensor.matmul(
                            ps[:],
                            lpT[:, c * SW + tt * 128: c * SW + (tt + 1) * 128],
                            w_kvp[:, c * 2 * C + C + half * 384: c * 2 * C + C + (half + 1) * 384],
                            start=(c == 0), stop=(c == CH - 1))
                    vtmp = psm.tile([128, 384], BF16, tag="vtmp", bufs=6)
                    nc.vector.tensor_copy(vtmp[:], ps[:])
                    for wl in range(2):
                        w = tt * 2 + wl
                        nc.gpsimd.dma_start(
                            vwr[0:64, w, half * 6:(half + 1) * 6, 0:64],
                            vtmp[wl * 64:(wl + 1) * 64, :].rearrange(
                                "p (h x) -> p h x", x=64))

            # windowed attention (k-major) + fused projection per 128-token
            # tile.  Per head: scores [128k, 64q/window]; exp output is the
            # AV stationary operand; AV emits token-major [q, hd | rowsum];
            # normalization folds into the psum evict.  AV is emitted one
            # head behind the scores to keep the PE queue deep.
            xiT = pstripe.tile([128, CH * SW], BF16, tag="xiT")
            for wp in range(4):
                gwp = s * 4 + wp
                wA, wB = 2 * wp, 2 * wp + 1

                M_sb = Mt_sb[:, wp * 128:(wp + 1) * 128]

                xi_tok = psm.tile([128, 768], BF16, tag="xitok", bufs=2)
                for half in range(2):
                    xw = ps_xw.tile([128, 390], F32, tag="xw")
                    pend = None
                    for h6 in range(6):
                        h = half * 6 + h6
                        m0, poff = (64 * h) // 128, (64 * h) % 128
                        sc = ps_sc.tile([128, 128], F32, tag="sc")
                        for wi, w in enumerate((wA, wB)):
                            nc.tensor.matmul(
                                sc[:, wi * 64:(wi + 1) * 64],
                                kw_all[poff:poff + 64, w * 768 + m0 * 128: w * 768 + (m0 + 1) * 128],
                                qT[poff:poff + 64, m0 * SW + w * 64: m0 * SW + (w + 1) * 64],
                                start=True, stop=True)
                        s2n = psm.tile([128, 128], BF16, tag="s2n", bufs=3)
                        nc.vector.tensor_tensor(out=s2n[:], in0=sc[:],
                                                in1=M_sb, op=ALU.min)
                        pexp = psm.tile([128, 128], BF16, tag="pexp", bufs=4)
                        nc.scalar.activation(pexp[:], s2n[:], AF.Exp, scale=1.0)

                        if pend is not None:
                            _emit_av(nc, psm, xw, xi_tok, vwr, *pend)
                        pend = (pexp, wA, wB, h, h6)
                    _emit_av(nc, psm, xw, xi_tok, vwr, *pend)

                for c in range(CH):
                    tp = ps_pt.tile([128, 128], BF16, tag="tp")
                    nc.tensor.transpose(tp[:], xi_tok[:, c * 128:(c + 1) * 128],
                                        ident[:])
                    if c % 2 == 0:
                        nc.scalar.copy(
                            xiT[:, c * SW + wp * 128: c * SW + (wp + 1) * 128], tp[:])
                    else:
                        nc.vector.tensor_copy(
                            xiT[:, c * SW + wp * 128: c * SW + (wp + 1) * 128], tp[:])

                for half in range(2):
                    y_ps = ps_g.tile([128, 384], F32, tag="g")
                    for c in range(CH):
                        nc.tensor.matmul(
                            y_ps[:], xiT[:, c * SW + wp * 128: c * SW + (wp + 1) * 128],
                            wproj[:, c * C + half * 384: c * C + (half + 1) * 384],
                            start=(c == 0), stop=(c == CH - 1))
                    y_sb = py.tile([128, 384], F32, tag="y_sb")
                    if half == 0:
                        nc.scalar.copy(y_sb[:], y_ps[:])
                    else:
                        nc.vector.tensor_copy(y_sb[:], y_ps[:])
                    t0 = s * SW + wp * 128
                    nc.sync.dma_start(
                        out_img_d[t0:t0 + 128, half * 384:(half + 1) * 384],
                        y_sb[:])

        if "b" not in PHASES:
            return nc
        # ---------------- tail: finish global aggregation ----------------
        sum_g = pB.tile([NH, 1], F32, tag="sum_g")
        nc.vector.tensor_reduce(out=sum_g[:], in_=pg_full[:], op=ALU.add,
                                axis=mybir.AxisListType.X)
        rinv_g = pB.tile([NH, 1], F32, tag="rinv_g")
        nc.vector.reciprocal(rinv_g[:], sum_g[:])

        xc_ps0 = ps_g.tile([NH, 384], F32, tag="g")
        xc_ps1 = ps_g.tile([NH, 384], F32, tag="g")
        xc_ps = [xc_ps0, xc_ps1]
        for t in range(32):
            vt = psm.tile([128, C], BF16, tag="bigload", bufs=6)
            q = nc.gpsimd if t % 2 == 0 else nc.sync
            q.dma_start(vt[:], v_spill[t * 128:(t + 1) * 128, :])
            for half in range(2):
                nc.tensor.matmul(
                    xc_ps[half][:], pgT_all[:, t * NH:(t + 1) * NH],
                    vt[:, half * 384:(half + 1) * 384],
                    start=(t == 0), stop=(t == 31))
        xcls_f = pB.tile([NH, C], F32, tag="xcls_f")
        for half in range(2):
            nc.vector.tensor_scalar_mul(xcls_f[:, half * 384:(half + 1) * 384],
                                        xc_ps[half][:], rinv_g[:])

        xcls_row = pB.tile([1, C], F32, tag="row_f32")
        for h in range(NH):
            nc.sync.dma_start(xcls_row[:, h * 64:(h + 1) * 64],
                              xcls_f[h:h + 1, h * 64:(h + 1) * 64])
        nc.sync.dma_start(cls_att_d[:], xcls_row[:])

    return nc


def _emit_av(nc, psm, xw, xi_tok, vwr, pexp, wA, wB, h, h6):
    for wi, w in enumerate((wA, wB)):
        nc.tensor.matmul(
            xw[wi * 64:(wi + 1) * 64, h6 * 65:(h6 + 1) * 65],
            pexp[:, wi * 64:(wi + 1) * 64],
            vwr[:, w, h, :],
            start=True, stop=True)
    rinv = psm.tile([128, 1], F32, tag="rinv", bufs=3)
    nc.vector.reciprocal(rinv[:], xw[:, h6 * 65 + 64: h6 * 65 + 65])
    nc.vector.tensor_scalar_mul(
        xi_tok[:, h * 64:(h + 1) * 64],
        xw[:, h6 * 65: h6 * 65 + 64], rinv[:])


_CACHE = {}


def _get_nc():
    if "nc" not in _CACHE:
        nc = build_nc()
        if not nc.is_finalized():
            nc.finalize()
        _CACHE["nc"] = nc
    return _CACHE["nc"]


def _window_major(a):
    # [B, 4096, ...] token order (i, a, j, b) -> window-major (i, j, a, b)
    s = a.shape[2:]
    return np.ascontiguousarray(
        a.reshape(B, 8, 8, 8, 8, *s)
        .transpose(0, 1, 3, 2, 4, *range(5, 5 + len(s)))
        .reshape(B, L, *s))


def _token_major(a):
    # inverse of _window_major for [L, ...] (single sample)
    s = a.shape[1:]
    return a.reshape(8, 8, 8, 8, *s).transpose(0, 2, 1, 3, *range(4, 4 + len(s))).reshape(L, *s)


def _featmaj(a_bf):
    # [L, C] window-major -> [128, ST*CH*SW] stripe-blocked feature-major
    return np.ascontiguousarray(
        a_bf.reshape(ST, SW, CH, 128).transpose(3, 0, 2, 1).reshape(128, ST * CH * SW))


def make_in_maps(x, mask, global_mask, layout_prefix, w_qkv, w_kv_prefix,
                 w_kv_global, w_proj, b_proj, H, W):
    bf = ml_dtypes.bfloat16
    x = np.asarray(x, np.float32)
    tok = np.ascontiguousarray(np.asarray(mask, np.float32)[:, :, 0])  # [B, L]
    gm = np.asarray(global_mask, np.float32).reshape(B, NH, L)
    mg = (2e4 * gm * tok[:, None, :] - 30.0).astype(bf)

    x_img_wm = _window_major(x[:, 1:].astype(bf))
    lp_wm = _window_major(np.asarray(layout_prefix, np.float32).astype(bf))
    tok_wm = _window_major(tok[..., None])[..., 0]          # [B, L] f32
    mg_wm = np.ascontiguousarray(
        _window_major(mg.transpose(0, 2, 1)).transpose(0, 2, 1))

    tk = tok_wm.reshape(B, 32, 2, 64)
    # M^T tiles [k, q] per window-pair: rows = [pre keys | img keys] (same
    # token mask), cols = [qA (vs A keys) | qB (vs B keys)]
    tkk = np.concatenate([tk, tk], axis=3)                  # [B, 32, 2, 128]
    Mt = 2e4 * tkk[..., None] * tk[..., None, :] - 30.0     # [B, 32, 2, 128, 64]
    Mt = Mt.transpose(0, 3, 1, 2, 4).reshape(B, 128, 32 * 128).astype(bf)

    wq_full = np.asarray(w_qkv, np.float32)
    wq_scaled = wq_full.copy()
    wq_scaled[:, :C] *= SCALE                 # exact (1/8 is a power of 2)
    wq = wq_scaled.astype(bf)
    wkp = np.asarray(w_kv_prefix, np.float32).astype(bf)
    wp = np.asarray(w_proj, np.float32).astype(bf)

    # host-precomputed global-query weights: sg = wsg^T @ x, where
    # wsg[:, h] = Wk[:, head h] @ (q_cls_h * scale)
    wk = wq_full[:, C:2 * C]                               # [C, C]
    in_maps = []
    for i in range(B):
        qc = (x[i, 0].astype(np.float64) @ wq_full[:, :C].astype(np.float64))
        qcs = (qc * SCALE).reshape(NH, HD)
        wsg = np.einsum('xhf,hf->xh', wk.reshape(C, NH, HD).astype(np.float64),
                        qcs)                                # [C, NH]
        wsg_dev = np.ascontiguousarray(
            wsg.astype(np.float32).astype(bf)
            .reshape(CH, 128, NH).transpose(1, 0, 2).reshape(128, CH * NH))
        in_maps.append(dict(
            xT=_featmaj(x_img_wm[i]), lpT=_featmaj(lp_wm[i]),
            Mt_all=np.ascontiguousarray(Mt[i]), mg=mg_wm[i], wsg=wsg_dev,
            w_qkv=wq, w_kvp=wkp, wproj=wp))
    return in_maps


def postprocess(results, x, w_kv_global, w_proj, b_proj):
    x = np.asarray(x, np.float32)
    wvg = np.asarray(w_kv_global, np.float32)[:, C:]        # [C, C] (V half)
    wp = np.asarray(w_proj, np.float32)
    bp = np.asarray(b_proj, np.float32)
    out = np.empty((B, N, C), np.float32)
    for i in range(B):
        cls_att = results[i]["cls_att"][0]                  # [C]
        out[i, 0] = cls_att @ wp + bp
        corr = (cls_att @ wvg) @ wp + bp                    # rank-1 row
        out[i, 1:] = _token_major(results[i]["out_img"]) + corr
    return out


def kernel(**inputs):
    nc = _get_nc()
    in_maps = make_in_maps(**inputs)
    from concourse.bass_utils import run_bass_kernel_spmd
    res = run_bass_kernel_spmd(nc, in_maps, core_ids=list(range(B)))
    return postprocess(res.results, inputs["x"], inputs["w_kv_global"],
                       inputs["w_proj"], inputs["b_proj"])
